# revision 9
# baseline (speedup 1.0000x reference)
"""EntityAttentionLayer on 8 Trainium2 NeuronCores.

Data-parallel over batch (16 batches/core). All matmuls bf16 with f32 PSUM
accumulation. Layouts avoid any PE transpose of activations: q/k are produced
pre-transposed by the projection layout, v naturally, and the attention output
is transposed by the DMA xbar on the two HWDGE queues.

The PE instruction stream is software-pipelined two batches deep so the
in-order PE queue never waits on the Scalar/Vector tail chains:

    per iteration i:
      k proj(i), v proj(i), [q proj at group start]
      logits(i) hc0 | attnv(i-1) h0-3 | logits hc1 | attnv(i-1) h4-7
      logits hc2 | logits hc3 | out proj(i-2)

The logits/attnv interleave also spaces out PSUM psl-bank reuse so the
serial exp chain on Scalar (the psl reader) is never on the PE critical path.

Math note: the reference computes
    w = softmax(logits masked with -inf); w[nan] = 0
    w = w * diff; w = w / (sum(w) + 1e-8)
which equals
    num = exp(logits) * valid * diff
    w   = num / (sum(num) + 1e-8 * sum(exp(logits) * valid))
Folding the 1e-8 into the mask: M = valid * (diff + 1e-8) gives
    w ~= exp(logits) * M / sum(exp(logits) * M)
with an O(1e-8) absolute perturbation on w (negligible vs bf16 rounding).
Fully-masked rows: numerator is exactly 0 and the denominator gets +1e-25,
so those rows come out exactly 0, matching the reference's NaN->0 path.
The post-mask zeroing and b_out add happen on the host after the gather.
"""

import numpy as np
import ml_dtypes

BS, NE, NQ = 128, 512, 128
DIN, EMB, ODIM = 512, 512, 512
H, HD = 8, 64
NCORES = 8
BPC = BS // NCORES          # batches per core
GRP = 4                     # batches per q-projection group
EC = DIN // 128             # contraction chunks (4)
BF16 = ml_dtypes.bfloat16


def _build_nc():
    import concourse.bacc as bacc
    import concourse.mybir as mybir
    import concourse.tile as tile
    from concourse.masks import make_identity

    f32 = mybir.dt.float32
    bf16 = mybir.dt.bfloat16

    nc = bacc.Bacc("TRN2", target_bir_lowering=False, debug=False,
                   num_devices=NCORES)

    ents_d = nc.dram_tensor("entsT", [BPC, DIN, NE], bf16, kind="ExternalInput")
    mask_d = nc.dram_tensor("maskT", [BPC, NE, NQ], bf16, kind="ExternalInput")
    win_d = nc.dram_tensor("w_inT", [DIN, 3 * EMB], bf16, kind="ExternalInput")
    wout_d = nc.dram_tensor("w_outT", [EMB, ODIM], bf16, kind="ExternalInput")
    out_d = nc.dram_tensor("out", [BPC, NQ, ODIM], bf16, kind="ExternalOutput")
    DBG = getattr(_build_nc, "DEBUG_TAPS", False)
    if DBG:
        dbg_kT = nc.dram_tensor("dbg_kT", [128, 4, NE], bf16, kind="ExternalOutput")
        dbg_qT = nc.dram_tensor("dbg_qT", [128, 4, GRP, NQ], bf16, kind="ExternalOutput")
        dbg_num = nc.dram_tensor("dbg_num", [128, 4, H, NQ], bf16, kind="ExternalOutput")
        dbg_v = nc.dram_tensor("dbg_v", [128, 4, H, HD + 1], bf16, kind="ExternalOutput")
        dbg_attn = nc.dram_tensor("dbg_attn", [128, EMB], bf16, kind="ExternalOutput")

    with tile.TileContext(nc) as tc:
        with (
            tc.tile_pool(name="const", bufs=1) as cpool,
            tc.tile_pool(name="gwork", bufs=2) as gwork,
            tc.tile_pool(name="work", bufs=2) as work,
            tc.tile_pool(name="work3", bufs=3) as work3,
            tc.tile_pool(name="nums", bufs=2) as nums,
            tc.tile_pool(name="exps", bufs=3) as exps,
            tc.tile_pool(name="ps", bufs=3, space="PSUM") as ps,
            tc.tile_pool(name="ps_l", bufs=3, space="PSUM") as ps_l,
            tc.tile_pool(name="ps_att", bufs=2, space="PSUM") as ps_att,
        ):
            # ---- constants (issue order matters: batch 0's k-projection
            # needs only w_in chunk 0 + the first entity slab) ----
            ident = cpool.tile([128, 128], bf16)
            make_identity(nc, ident)
            w_in_sb = cpool.tile([128, EC, 3 * EMB], bf16)
            win_r = win_d.ap().rearrange("(c p) f -> p c f", p=128)
            nc.sync.dma_start(out=w_in_sb[:, 0, :], in_=win_r[:, 0, :])
            w_out_sb = cpool.tile([128, EC, ODIM], bf16)

            def late_consts():
                for ce in range(1, EC):
                    nc.sync.dma_start(out=w_in_sb[:, ce, :], in_=win_r[:, ce, :])
                nc.sync.dma_start(
                    out=w_out_sb,
                    in_=wout_d.ap().rearrange("(c p) f -> p c f", p=128))

            # warm-up matmuls: engage the PE HAM while the first weight and
            # entity DMAs are in flight so real matmuls start at full clock
            psum_w = ps.tile([128, 128], f32, tag="big", name="psum_w")
            for _ in range(22):
                nc.tensor.matmul(psum_w, lhsT=ident, rhs=ident,
                                 start=True, stop=True)

            def load_group(g):
                ents_sb = gwork.tile([128, EC, GRP, NE], bf16, name="ents_sb")
                for i in range(GRP):
                    nc.sync.dma_start(
                        out=ents_sb[:, :, i, :],
                        in_=ents_d.ap()[g * GRP + i]
                            .rearrange("(c p) n -> p c n", p=128))
                    if g == 0 and i == 0:
                        late_consts()
                return ents_sb

            def load_mask(b):
                mask_sb = work3.tile([128, EC, NQ], bf16, name="mask_sb")
                nc.sync.dma_start(
                    out=mask_sb,
                    in_=mask_d.ap()[b].rearrange("(c p) q -> p c q", p=128))
                return mask_sb

            def qproj(ents_sb):
                # fused qT projection for the whole group:
                # qT4[f, i, q] , i = batch within group
                qT_sb = gwork.tile([128, 4, GRP, NQ], bf16, name="qT_sb")
                for cf in range(4):
                    psum_q = ps.tile([128, GRP, NQ], f32, tag="big",
                                     name="psum_q")
                    for ce in range(EC):
                        nc.tensor.matmul(
                            psum_q,
                            lhsT=w_in_sb[:, ce, 128 * cf:128 * (cf + 1)],
                            rhs=ents_sb[:, ce, :, 0:NQ],
                            start=(ce == 0), stop=(ce == EC - 1))
                    if cf % 2 == 0:
                        nc.scalar.copy(qT_sb[:, cf, :, :], psum_q)
                    else:
                        nc.vector.tensor_copy(qT_sb[:, cf, :, :], psum_q)
                return qT_sb

            def kproj(b, ents_sb, i):
                """kT projection for batch b (slot i in group): kT[f, n]."""
                kT_sb = work.tile([128, 4, NE], bf16, name="kT_sb")
                for cf in range(4):
                    psum_k = ps.tile([128, NE], f32, tag="big", name="psum_k")
                    for ce in range(EC):
                        nc.tensor.matmul(
                            psum_k,
                            lhsT=w_in_sb[:, ce, EMB + 128 * cf:EMB + 128 * (cf + 1)],
                            rhs=ents_sb[:, ce, i, :],
                            start=(ce == 0), stop=(ce == EC - 1))
                    nc.vector.tensor_copy(kT_sb[:, cf, :], psum_k)
                return kT_sb

            def vproj(b, ents_sb, i):
                """v projection (natural layout) + ones column."""
                v_sb = work.tile([128, 4, H, HD + 1], bf16, name="v_sb")
                nc.gpsimd.memset(v_sb[:, :, :, HD], 1.0)
                for cn in range(4):
                    psum_v = ps.tile([128, EMB], f32, tag="big", name="psum_v")
                    for ce in range(EC):
                        nc.tensor.matmul(
                            psum_v,
                            lhsT=ents_sb[:, ce, i, 128 * cn:128 * (cn + 1)],
                            rhs=w_in_sb[:, ce, 2 * EMB:3 * EMB],
                            start=(ce == 0), stop=(ce == EC - 1))
                    src = psum_v.rearrange("p (h d) -> p h d", h=H)
                    nc.vector.tensor_copy(v_sb[:, cn, :, 0:HD], src)
                return v_sb

            def logits_pair(hc, qT_sb, i, kT_sb, mask_sb, num_sb):
                """logits + exp + mask for head pair (2*hc, 2*hc+1)."""
                psl = [
                    ps_l.tile([128, 4, NQ], f32, tag="psl", name="psl0"),
                    ps_l.tile([128, 4, NQ], f32, tag="psl", name="psl1"),
                ]
                for cn in range(4):
                    for r in range(2):   # row group r*64
                        nc.tensor.matmul(
                            psl[r][:, cn, :],
                            lhsT=kT_sb[64 * r:64 * (r + 1), hc,
                                       128 * cn:128 * (cn + 1)],
                            rhs=qT_sb[64 * r:64 * (r + 1), hc, i, :],
                            start=True, stop=True)
                for r in range(2):
                    h = 2 * hc + r
                    exp_sb = exps.tile([128, 4, NQ], bf16, tag="exp",
                                       name="exp_sb")
                    nc.scalar.activation(
                        exp_sb, psl[r],
                        mybir.ActivationFunctionType.Exp, scale=1.0 / 8.0)
                    nc.gpsimd.tensor_mul(num_sb[:, :, h, :], exp_sb, mask_sb)

            def attnv_half(j, num_sb, v_sb, att_tiles):
                """attention @ v for heads 4j .. 4j+3."""
                patt = att_tiles[j]
                for hj in range(4):
                    h = 4 * j + hj
                    for cn in range(4):
                        nc.tensor.matmul(
                            patt[:, hj, :],
                            lhsT=num_sb[:, cn, h, :],
                            rhs=v_sb[:, cn, h, :],
                            start=(cn == 0), stop=(cn == 3))

            def tail_v(att_tiles):
                """denominators + fused normalize (Vector only)."""
                deps_sb = work.tile([128, H], f32, name="deps_sb")
                nc.vector.tensor_scalar_add(
                    deps_sb[:, 0:4], att_tiles[0][:, :, HD], 1e-25)
                nc.vector.tensor_scalar_add(
                    deps_sb[:, 4:8], att_tiles[1][:, :, HD], 1e-25)
                recip_sb = work.tile([128, H], f32, name="recip_sb")
                nc.vector.reciprocal(recip_sb, deps_sb)

                attn_sb = work.tile([128, EMB], bf16, name="attn_sb")
                for j in range(2):
                    nc.vector.tensor_mul(
                        attn_sb[:, 256 * j:256 * (j + 1)]
                            .rearrange("p (g d) -> p g d", g=4),
                        att_tiles[j][:, :, 0:HD],
                        recip_sb[:, 4 * j:4 * (j + 1)]
                            .unsqueeze(2).broadcast_to([128, 4, HD]))
                return attn_sb

            def transpose_attn(attn_sb):
                """attn -> attnT[E, q] on the DMA xbar, 2 per HWDGE queue."""
                attnT_sb = work3.tile([128, 4, 128], bf16, name="attnT_sb")
                for ct in range(4):
                    eng = nc.sync if ct < 2 else nc.scalar
                    eng.dma_start_transpose(
                        attnT_sb[:, ct, :], attn_sb[:, 128 * ct:128 * (ct + 1)])
                return attnT_sb

            def outproj(b, attnT_sb):
                psum_o = ps.tile([128, ODIM], f32, tag="big", name="psum_o")
                for ct in range(4):
                    nc.tensor.matmul(
                        psum_o,
                        lhsT=attnT_sb[:, ct, :],
                        rhs=w_out_sb[:, ct, :],
                        start=(ct == 0), stop=(ct == 3))
                out_sb = work3.tile([128, ODIM], bf16, name="out_sb")
                nc.vector.tensor_copy(out_sb, psum_o)
                nc.sync.dma_start(out=out_d.ap()[b], in_=out_sb)

            ngrp = BPC // GRP
            ents_cur = load_group(0)
            mask_cur = load_mask(0)
            qT_sb = None
            # pipeline state: (num, v) of batch i-1, att tiles of batch i-1,
            # attn_sb of i-1, attnT of i-2
            prev_nv = None
            prev_attnT = [None, None]   # [i-2, i-1] transposed attn
            for b in range(BPC):
                g, i = divmod(b, GRP)
                kT_sb = kproj(b, ents_cur, i)
                v_sb = vproj(b, ents_cur, i)
                if DBG and b == 3:
                    nc.sync.dma_start(out=dbg_kT.ap(), in_=kT_sb)
                    nc.sync.dma_start(out=dbg_v.ap(), in_=v_sb)
                if i == 0:
                    qT_sb = qproj(ents_cur)
                    if DBG and b == 0:
                        nc.sync.dma_start(out=dbg_qT.ap(), in_=qT_sb)
                mask_b = mask_cur
                if b + 1 < BPC:
                    mask_cur = load_mask(b + 1)
                ents_next = ents_cur
                if i == 3 and g + 1 < ngrp:
                    ents_next = load_group(g + 1)

                num_sb = nums.tile([128, 4, H, NQ], bf16, name="num_sb")
                att_tiles = [
                    ps_att.tile([128, 4, HD + 1], f32, tag="att", name="patt0"),
                    ps_att.tile([128, 4, HD + 1], f32, tag="att", name="patt1"),
                ]

                INTERLEAVE = True
                if INTERLEAVE:
                    logits_pair(0, qT_sb, i, kT_sb, mask_b, num_sb)
                    if prev_nv is not None:
                        attnv_half(0, prev_nv[0], prev_nv[1], prev_att)
                    logits_pair(1, qT_sb, i, kT_sb, mask_b, num_sb)
                    if prev_nv is not None:
                        attnv_half(1, prev_nv[0], prev_nv[1], prev_att)
                    logits_pair(2, qT_sb, i, kT_sb, mask_b, num_sb)
                    if prev_nv is not None:
                        attn_sb = tail_v(prev_att)
                        prev_attnT[1] = transpose_attn(attn_sb)
                    logits_pair(3, qT_sb, i, kT_sb, mask_b, num_sb)
                    if prev_attnT[0] is not None:
                        outproj(b - 2, prev_attnT[0])
                else:
                    for hc in range(4):
                        logits_pair(hc, qT_sb, i, kT_sb, mask_b, num_sb)
                    if prev_nv is not None:
                        attnv_half(0, prev_nv[0], prev_nv[1], prev_att)
                        attnv_half(1, prev_nv[0], prev_nv[1], prev_att)
                        if DBG and b == 4:
                            nc.sync.dma_start(out=dbg_num.ap(), in_=prev_nv[0])
                        attn_sb = tail_v(prev_att)
                        if DBG and b == 4:
                            nc.sync.dma_start(out=dbg_attn.ap(), in_=attn_sb)
                        prev_attnT[1] = transpose_attn(attn_sb)
                    if prev_attnT[0] is not None:
                        outproj(b - 2, prev_attnT[0])

                prev_nv = (num_sb, v_sb)
                prev_att = att_tiles
                prev_attnT[0] = prev_attnT[1]
                ents_cur = ents_next

            # epilogue: batch 15's attention + the last two out projections
            attnv_half(0, prev_nv[0], prev_nv[1], prev_att)
            attnv_half(1, prev_nv[0], prev_nv[1], prev_att)
            attn_sb = tail_v(prev_att)
            attnT_last = transpose_attn(attn_sb)
            outproj(BPC - 2, prev_attnT[0])
            # keep the PE HAM engaged while the last transpose drains
            psum_w2 = ps.tile([128, 128], f32, tag="big", name="psum_w2")
            for _ in range(10):
                nc.tensor.matmul(psum_w2, lhsT=ident, rhs=ident,
                                 start=True, stop=True)
            outproj(BPC - 1, attnT_last)

    nc.compile()
    return nc


def _prep_inputs(entities, pre_mask, diff_mask, W_in, W_out):
    entities = np.asarray(entities, dtype=np.float32)
    pre_mask = np.asarray(pre_mask, dtype=bool)
    diff_mask = np.asarray(diff_mask, dtype=np.float32)
    W_in = np.asarray(W_in, dtype=np.float32)
    W_out = np.asarray(W_out, dtype=np.float32)

    entsT = np.ascontiguousarray(entities.transpose(0, 2, 1)).astype(BF16)
    m = (~pre_mask).astype(np.float32) * (diff_mask + 1e-8)
    maskT = np.ascontiguousarray(m.transpose(0, 2, 1)).astype(BF16)
    w_inT = np.ascontiguousarray(W_in.T).astype(BF16)
    w_outT = np.ascontiguousarray(W_out.T).astype(BF16)

    in_maps = []
    for c in range(NCORES):
        sl = slice(c * BPC, (c + 1) * BPC)
        in_maps.append({
            "entsT": np.ascontiguousarray(entsT[sl]),
            "maskT": np.ascontiguousarray(maskT[sl]),
            "w_inT": w_inT,
            "w_outT": w_outT,
        })
    return in_maps


def _run(in_maps, trace=False):
    from concourse.bass_utils import run_bass_kernel_spmd
    nc = _build_nc()
    last_exc = None
    for attempt in range(3):
        try:
            return run_bass_kernel_spmd(
                nc, in_maps, core_ids=list(range(NCORES)), trace=trace)
        except Exception as e:  # transient NRT_EXEC_UNIT faults on fresh NEFFs
            last_exc = e
            import time
            time.sleep(2.0 * (attempt + 1))
    raise last_exc


def kernel_traced(entities, pre_mask, diff_mask, post_mask, W_in, W_out, b_out,
                  trace=False):
    """Returns (output, BassKernelResults)."""
    b_out = np.asarray(b_out, dtype=np.float32)
    post_mask_np = np.asarray(post_mask, dtype=bool)
    in_maps = _prep_inputs(entities, pre_mask, diff_mask, W_in, W_out)
    res = _run(in_maps, trace=trace)
    out = np.concatenate([r["out"] for r in res.results], axis=0)
    out = out.astype(np.float32) + b_out[None, None, :]
    out = np.where(post_mask_np[:, :, None], 0.0, out)
    return out.astype(np.float32), res


def kernel(entities, pre_mask, diff_mask, post_mask, W_in, W_out, b_out):
    out, _ = kernel_traced(entities, pre_mask, diff_mask, post_mask,
                           W_in, W_out, b_out)
    return out


# revision 11
# speedup vs baseline: 1.0960x; 1.0960x over previous
"""EntityAttentionLayer on 8 Trainium2 NeuronCores.

Data-parallel over batch (16 batches/core). All matmuls bf16 with f32 PSUM
accumulation. Layouts avoid any PE transpose of activations: q/k are produced
pre-transposed by the projection layout, v naturally, and the attention output
is transposed by the DMA xbar on the two HWDGE queues.

The PE instruction stream is software-pipelined two batches deep so the
in-order PE queue never waits on the Scalar/Vector tail chains:

    per iteration i:
      k proj(i), v proj(i), [q proj at group start]
      logits(i) hc0 | attnv(i-1) h0-3 | logits hc1 | attnv(i-1) h4-7
      logits hc2 | logits hc3 | out proj(i-2)

The logits/attnv interleave also spaces out PSUM psl-bank reuse so the
serial exp chain on Scalar (the psl reader) is never on the PE critical path.

Math note: the reference computes
    w = softmax(logits masked with -inf); w[nan] = 0
    w = w * diff; w = w / (sum(w) + 1e-8)
which equals
    num = exp(logits) * valid * diff
    w   = num / (sum(num) + 1e-8 * sum(exp(logits) * valid))
Folding the 1e-8 into the mask: M = valid * (diff + 1e-8) gives
    w ~= exp(logits) * M / sum(exp(logits) * M)
with an O(1e-8) absolute perturbation on w (negligible vs bf16 rounding).
Fully-masked rows: numerator is exactly 0 and the denominator gets +1e-25,
so those rows come out exactly 0, matching the reference's NaN->0 path.
The post-mask zeroing and b_out add happen on the host after the gather.
"""

import numpy as np
import ml_dtypes

BS, NE, NQ = 128, 512, 128
DIN, EMB, ODIM = 512, 512, 512
H, HD = 8, 64
NCORES = 8
BPC = BS // NCORES          # batches per core
GRP = 4                     # batches per q-projection group
EC = DIN // 128             # contraction chunks (4)
BF16 = ml_dtypes.bfloat16


def _build_nc():
    import concourse.bacc as bacc
    import concourse.mybir as mybir
    import concourse.tile as tile
    from concourse.masks import make_identity

    f32 = mybir.dt.float32
    bf16 = mybir.dt.bfloat16

    nc = bacc.Bacc("TRN2", target_bir_lowering=False, debug=False,
                   num_devices=NCORES)

    ents_d = nc.dram_tensor("entsT", [BPC, DIN, NE], bf16, kind="ExternalInput")
    mask_d = nc.dram_tensor("maskT", [BPC, NE, NQ], bf16, kind="ExternalInput")
    win_d = nc.dram_tensor("w_inT", [DIN, 3 * EMB], bf16, kind="ExternalInput")
    wout_d = nc.dram_tensor("w_outT", [EMB, ODIM], bf16, kind="ExternalInput")
    out_d = nc.dram_tensor("out", [BPC, NQ, ODIM], bf16, kind="ExternalOutput")
    DBG = getattr(_build_nc, "DEBUG_TAPS", False)
    if DBG:
        dbg_kT = nc.dram_tensor("dbg_kT", [128, 4, NE], bf16, kind="ExternalOutput")
        dbg_qT = nc.dram_tensor("dbg_qT", [128, 4, GRP, NQ], bf16, kind="ExternalOutput")
        dbg_num = nc.dram_tensor("dbg_num", [128, 4, H, NQ], bf16, kind="ExternalOutput")
        dbg_v = nc.dram_tensor("dbg_v", [128, 4, H, HD + 1], bf16, kind="ExternalOutput")
        dbg_attn = nc.dram_tensor("dbg_attn", [128, EMB], bf16, kind="ExternalOutput")

    with tile.TileContext(nc) as tc:
        with (
            tc.tile_pool(name="const", bufs=1) as cpool,
            tc.tile_pool(name="pents", bufs=2) as pents,
            tc.tile_pool(name="pqt", bufs=2) as pqt,
            tc.tile_pool(name="work", bufs=2) as work,
            tc.tile_pool(name="work3", bufs=3) as work3,
            tc.tile_pool(name="nums", bufs=2) as nums,
            tc.tile_pool(name="exps", bufs=3) as exps,
            tc.tile_pool(name="ps", bufs=6, space="PSUM") as ps,
            tc.tile_pool(name="ps_att", bufs=2, space="PSUM") as ps_att,
        ):
            # ---- constants (issue order matters: batch 0's k-projection
            # needs only w_in chunk 0 + the first entity slab) ----
            ident = cpool.tile([128, 128], bf16)
            make_identity(nc, ident)
            w_in_sb = cpool.tile([128, EC, 3 * EMB], bf16)
            win_r = win_d.ap().rearrange("(c p) f -> p c f", p=128)
            nc.sync.dma_start(out=w_in_sb[:, 0, :], in_=win_r[:, 0, :])
            w_out_sb = cpool.tile([128, EC, ODIM], bf16)

            def late_consts():
                for ce in range(1, EC):
                    nc.sync.dma_start(out=w_in_sb[:, ce, :], in_=win_r[:, ce, :])
                nc.sync.dma_start(
                    out=w_out_sb,
                    in_=wout_d.ap().rearrange("(c p) f -> p c f", p=128))

            # warm-up matmuls: engage the PE HAM while the first weight and
            # entity DMAs are in flight so real matmuls start at full clock
            psum_w = ps.tile([128, 128], f32, tag="big", name="psum_w")
            for _ in range(22):
                nc.tensor.matmul(psum_w, lhsT=ident, rhs=ident,
                                 start=True, stop=True)

            def load_group(g):
                ents_sb = pents.tile([128, EC, GRP, NE], bf16, name="ents_sb")
                for i in range(GRP):
                    nc.sync.dma_start(
                        out=ents_sb[:, :, i, :],
                        in_=ents_d.ap()[g * GRP + i]
                            .rearrange("(c p) n -> p c n", p=128))
                    if g == 0 and i == 0:
                        late_consts()
                return ents_sb

            def load_mask(b):
                mask_sb = work3.tile([128, EC, NQ], bf16, name="mask_sb")
                nc.sync.dma_start(
                    out=mask_sb,
                    in_=mask_d.ap()[b].rearrange("(c p) q -> p c q", p=128))
                return mask_sb

            def qproj(ents_sb):
                # fused qT projection for the whole group:
                # qT4[f, i, q] , i = batch within group
                qT_sb = pqt.tile([128, 4, GRP, NQ], bf16, name="qT_sb")
                for cf in range(4):
                    psum_q = ps.tile([128, GRP, NQ], f32, tag="big",
                                     name="psum_q")
                    for ce in range(EC):
                        nc.tensor.matmul(
                            psum_q,
                            lhsT=w_in_sb[:, ce, 128 * cf:128 * (cf + 1)],
                            rhs=ents_sb[:, ce, :, 0:NQ],
                            start=(ce == 0), stop=(ce == EC - 1))
                    if cf % 2 == 0:
                        nc.scalar.copy(qT_sb[:, cf, :, :], psum_q)
                    else:
                        nc.vector.tensor_copy(qT_sb[:, cf, :, :], psum_q)
                return qT_sb

            def kproj(b, ents_sb, i):
                """kT projection for batch b (slot i in group): kT[f, n]."""
                kT_sb = work.tile([128, 4, NE], bf16, name="kT_sb")
                for cf in range(4):
                    psum_k = ps.tile([128, NE], f32, tag="big", name="psum_k")
                    for ce in range(EC):
                        nc.tensor.matmul(
                            psum_k,
                            lhsT=w_in_sb[:, ce, EMB + 128 * cf:EMB + 128 * (cf + 1)],
                            rhs=ents_sb[:, ce, i, :],
                            start=(ce == 0), stop=(ce == EC - 1))
                    nc.vector.tensor_copy(kT_sb[:, cf, :], psum_k)
                return kT_sb

            def vproj(b, ents_sb, i):
                """v projection (natural layout) + ones column."""
                v_sb = work.tile([128, 4, H, HD + 1], bf16, name="v_sb")
                nc.gpsimd.memset(v_sb[:, :, :, HD], 1.0)
                for cn in range(4):
                    psum_v = ps.tile([128, EMB], f32, tag="big", name="psum_v")
                    for ce in range(EC):
                        nc.tensor.matmul(
                            psum_v,
                            lhsT=ents_sb[:, ce, i, 128 * cn:128 * (cn + 1)],
                            rhs=w_in_sb[:, ce, 2 * EMB:3 * EMB],
                            start=(ce == 0), stop=(ce == EC - 1))
                    src = psum_v.rearrange("p (h d) -> p h d", h=H)
                    nc.vector.tensor_copy(v_sb[:, cn, :, 0:HD], src)
                return v_sb

            def logits_pair(hc, qT_sb, i, kT_sb, mask_sb, num_sb):
                """logits + exp + mask for head pair (2*hc, 2*hc+1)."""
                psl = [
                    ps.tile([128, 4, NQ], f32, tag="big", name="psl0"),
                    ps.tile([128, 4, NQ], f32, tag="big", name="psl1"),
                ]
                for cn in range(4):
                    for r in range(2):   # row group r*64
                        nc.tensor.matmul(
                            psl[r][:, cn, :],
                            lhsT=kT_sb[64 * r:64 * (r + 1), hc,
                                       128 * cn:128 * (cn + 1)],
                            rhs=qT_sb[64 * r:64 * (r + 1), hc, i, :],
                            start=True, stop=True)
                for r in range(2):
                    h = 2 * hc + r
                    exp_sb = exps.tile([128, 4, NQ], bf16, tag="exp",
                                       name="exp_sb")
                    nc.scalar.activation(
                        exp_sb, psl[r],
                        mybir.ActivationFunctionType.Exp, scale=1.0 / 8.0)
                    nc.gpsimd.tensor_mul(num_sb[:, :, h, :], exp_sb, mask_sb)

            def attnv_half(j, num_sb, v_sb, att_tiles):
                """attention @ v for heads 4j .. 4j+3."""
                patt = att_tiles[j]
                for hj in range(4):
                    h = 4 * j + hj
                    for cn in range(4):
                        nc.tensor.matmul(
                            patt[:, hj, :],
                            lhsT=num_sb[:, cn, h, :],
                            rhs=v_sb[:, cn, h, :],
                            start=(cn == 0), stop=(cn == 3))

            def tail_v(att_tiles):
                """denominators + fused normalize (Vector only)."""
                deps_sb = work.tile([128, H], f32, name="deps_sb")
                nc.vector.tensor_scalar_add(
                    deps_sb[:, 0:4], att_tiles[0][:, :, HD], 1e-25)
                nc.vector.tensor_scalar_add(
                    deps_sb[:, 4:8], att_tiles[1][:, :, HD], 1e-25)
                recip_sb = work.tile([128, H], f32, name="recip_sb")
                nc.vector.reciprocal(recip_sb, deps_sb)

                attn_sb = work.tile([128, EMB], bf16, name="attn_sb")
                for j in range(2):
                    nc.vector.tensor_mul(
                        attn_sb[:, 256 * j:256 * (j + 1)]
                            .rearrange("p (g d) -> p g d", g=4),
                        att_tiles[j][:, :, 0:HD],
                        recip_sb[:, 4 * j:4 * (j + 1)]
                            .unsqueeze(2).broadcast_to([128, 4, HD]))
                return attn_sb

            def transpose_attn(attn_sb):
                """attn -> attnT[E, q] on the DMA xbar, 2 per HWDGE queue."""
                attnT_sb = work3.tile([128, 4, 128], bf16, name="attnT_sb")
                for ct in range(4):
                    eng = nc.sync if ct < 2 else nc.scalar
                    eng.dma_start_transpose(
                        attnT_sb[:, ct, :], attn_sb[:, 128 * ct:128 * (ct + 1)])
                return attnT_sb

            def outproj(b, attnT_sb):
                psum_o = ps.tile([128, ODIM], f32, tag="big", name="psum_o")
                for ct in range(4):
                    nc.tensor.matmul(
                        psum_o,
                        lhsT=attnT_sb[:, ct, :],
                        rhs=w_out_sb[:, ct, :],
                        start=(ct == 0), stop=(ct == 3))
                out_sb = work3.tile([128, ODIM], bf16, name="out_sb")
                nc.vector.tensor_copy(out_sb, psum_o)
                nc.sync.dma_start(out=out_d.ap()[b], in_=out_sb)

            ngrp = BPC // GRP
            ents_cur = load_group(0)
            mask_cur = load_mask(0)
            qT_sb = None
            # pipeline state: (num, v) of batch i-1, att tiles of batch i-1,
            # attn_sb of i-1, attnT of i-2
            prev_nv = None
            prev_attnT = [None, None]   # [i-2, i-1] transposed attn
            for b in range(BPC):
                g, i = divmod(b, GRP)
                kT_sb = kproj(b, ents_cur, i)
                v_sb = vproj(b, ents_cur, i)
                if DBG and b == 3:
                    nc.sync.dma_start(out=dbg_kT.ap(), in_=kT_sb)
                    nc.sync.dma_start(out=dbg_v.ap(), in_=v_sb)
                if i == 0:
                    qT_sb = qproj(ents_cur)
                    if DBG and b == 0:
                        nc.sync.dma_start(out=dbg_qT.ap(), in_=qT_sb)
                mask_b = mask_cur
                if b + 1 < BPC:
                    mask_cur = load_mask(b + 1)
                if i == 2 and g + 1 < ngrp:
                    ents_pending = load_group(g + 1)

                num_sb = nums.tile([128, 4, H, NQ], bf16, name="num_sb")
                att_tiles = [
                    ps_att.tile([128, 4, HD + 1], f32, tag="att", name="patt0"),
                    ps_att.tile([128, 4, HD + 1], f32, tag="att", name="patt1"),
                ]

                INTERLEAVE = True
                if INTERLEAVE:
                    logits_pair(0, qT_sb, i, kT_sb, mask_b, num_sb)
                    if prev_nv is not None:
                        attnv_half(0, prev_nv[0], prev_nv[1], prev_att)
                    logits_pair(1, qT_sb, i, kT_sb, mask_b, num_sb)
                    if prev_nv is not None:
                        attnv_half(1, prev_nv[0], prev_nv[1], prev_att)
                    logits_pair(2, qT_sb, i, kT_sb, mask_b, num_sb)
                    if prev_nv is not None:
                        attn_sb = tail_v(prev_att)
                        prev_attnT[1] = transpose_attn(attn_sb)
                    logits_pair(3, qT_sb, i, kT_sb, mask_b, num_sb)
                    if prev_attnT[0] is not None:
                        outproj(b - 2, prev_attnT[0])
                else:
                    for hc in range(4):
                        logits_pair(hc, qT_sb, i, kT_sb, mask_b, num_sb)
                    if prev_nv is not None:
                        attnv_half(0, prev_nv[0], prev_nv[1], prev_att)
                        attnv_half(1, prev_nv[0], prev_nv[1], prev_att)
                        if DBG and b == 4:
                            nc.sync.dma_start(out=dbg_num.ap(), in_=prev_nv[0])
                        attn_sb = tail_v(prev_att)
                        if DBG and b == 4:
                            nc.sync.dma_start(out=dbg_attn.ap(), in_=attn_sb)
                        prev_attnT[1] = transpose_attn(attn_sb)
                    if prev_attnT[0] is not None:
                        outproj(b - 2, prev_attnT[0])

                prev_nv = (num_sb, v_sb)
                prev_att = att_tiles
                prev_attnT[0] = prev_attnT[1]
                if i == 3:
                    ents_cur = ents_pending

            # epilogue: batch 15's attention + the last two out projections
            attnv_half(0, prev_nv[0], prev_nv[1], prev_att)
            attnv_half(1, prev_nv[0], prev_nv[1], prev_att)
            attn_sb = tail_v(prev_att)
            attnT_last = transpose_attn(attn_sb)
            outproj(BPC - 2, prev_attnT[0])
            # keep the PE HAM engaged while the last transpose drains
            psum_w2 = ps.tile([128, 128], f32, tag="big", name="psum_w2")
            for _ in range(10):
                nc.tensor.matmul(psum_w2, lhsT=ident, rhs=ident,
                                 start=True, stop=True)
            outproj(BPC - 1, attnT_last)

    nc.compile()
    return nc


def _prep_inputs(entities, pre_mask, diff_mask, W_in, W_out):
    entities = np.asarray(entities, dtype=np.float32)
    pre_mask = np.asarray(pre_mask, dtype=bool)
    diff_mask = np.asarray(diff_mask, dtype=np.float32)
    W_in = np.asarray(W_in, dtype=np.float32)
    W_out = np.asarray(W_out, dtype=np.float32)

    entsT = np.ascontiguousarray(entities.transpose(0, 2, 1)).astype(BF16)
    m = (~pre_mask).astype(np.float32) * (diff_mask + 1e-8)
    maskT = np.ascontiguousarray(m.transpose(0, 2, 1)).astype(BF16)
    w_inT = np.ascontiguousarray(W_in.T).astype(BF16)
    w_outT = np.ascontiguousarray(W_out.T).astype(BF16)

    in_maps = []
    for c in range(NCORES):
        sl = slice(c * BPC, (c + 1) * BPC)
        in_maps.append({
            "entsT": np.ascontiguousarray(entsT[sl]),
            "maskT": np.ascontiguousarray(maskT[sl]),
            "w_inT": w_inT,
            "w_outT": w_outT,
        })
    return in_maps


def _run(in_maps, trace=False):
    from concourse.bass_utils import run_bass_kernel_spmd
    nc = _build_nc()
    last_exc = None
    for attempt in range(3):
        try:
            return run_bass_kernel_spmd(
                nc, in_maps, core_ids=list(range(NCORES)), trace=trace)
        except Exception as e:  # transient NRT_EXEC_UNIT faults on fresh NEFFs
            last_exc = e
            import time
            time.sleep(2.0 * (attempt + 1))
    raise last_exc


def kernel_traced(entities, pre_mask, diff_mask, post_mask, W_in, W_out, b_out,
                  trace=False):
    """Returns (output, BassKernelResults)."""
    b_out = np.asarray(b_out, dtype=np.float32)
    post_mask_np = np.asarray(post_mask, dtype=bool)
    in_maps = _prep_inputs(entities, pre_mask, diff_mask, W_in, W_out)
    res = _run(in_maps, trace=trace)
    out = np.concatenate([r["out"] for r in res.results], axis=0)
    out = out.astype(np.float32) + b_out[None, None, :]
    out = np.where(post_mask_np[:, :, None], 0.0, out)
    return out.astype(np.float32), res


def kernel(entities, pre_mask, diff_mask, post_mask, W_in, W_out, b_out):
    out, _ = kernel_traced(entities, pre_mask, diff_mask, post_mask,
                           W_in, W_out, b_out)
    return out
